# revision 49
# baseline (speedup 1.0000x reference)
"""Adaptive thresholding (11x11 box mean, BORDER_REPLICATE, THRESH_BINARY_INV)
on 8 TRN2 NeuronCores, data-parallel over the batch dim (16 images/core).

v6.5 design (PE-saturated schedule; ~64us vs the 70us v5 baseline):
  - Host pre-bakes, per image, a [128, 4, 533] fp16 plane holding y = fp16(x)/4
    with an 11-col zero head and 5-col replicate margins per segment.
  - Custom DVE op ADAPT_WSCAN: out = inclusive_scan_add(Src0 - Src1) computes
    the horizontal 11-window sums s in one 1-elem/cycle pass per image.
  - Input DMA uses the flat [128, 4264B-contiguous] view (128 big descriptors
    instead of 512 small ones) so the input stream outpaces the pipeline.
  - Vertical 11-window sum on PE: 4 band matmuls (bm_top/mid/bot) + 6 halo
    matmuls (bhn/bhp) per image. (Compact-halo via SBUF->SBUF gathers was
    tried and abandoned: 3 small DMAs/image cost more queue time than the 2
    matmuls they save, and the Tile scheduler then serializes around them.)
  - x-injection (-121*y into PSUM): idn matmuls, except bank 3 of EVERY
    image and bank 2 of every third image are pre-loaded by the Scalar
    engine (ACT Copy with scale=-121 into PSUM) and those banks' matmuls
    accumulate via start=False + skip_group_check. This keeps PE the sole
    bottleneck (99% duty, all matmuls at the full-clock 216ns) with ACT at
    ~87% duty - the equilibrium of the two engines.
  - PSUM is managed as half-image tiles ([128, 2, 512], 4 bufs); each half
    gets its own Sign, halving PSUM residency so PE never stalls on buffer
    recycling (stalls drop the PE p-state clock 2x).
  - PE warmup: 8 dummy matmuls cycle a start/stop group through all 8 psum
    banks before real work: ramps the clock AND consumes power-on lazy-zero
    ("pending zero") psum state, which would otherwise corrupt the first
    start=False accumulate onto ACT-written psum (first-run-only flake).
  - out-DMA per image on the idle gpsimd/SWDGE queue; drain (img 15) stores
    per half on sync; fill (img 0) uses chunked input DMA + per-seg scans.
Host: out = (sign >= 0) * 255.
"""
import sys
sys.path.insert(0, '/opt/trn_rl_repo')
import numpy as np
import concourse.bass as bass
import concourse.tile as tile
from concourse import bacc, mybir
from concourse.bass_utils import run_bass_kernel_spmd
from concourse import dve_ops as _dops
from concourse.dve_spec import Spec, Src0, Src1, scan, AluOp, lower
from concourse.dve_spec import _has_src1 as _hs1
from concourse.dve_uop import DveOpSpec

F32 = mybir.dt.float32
F16 = mybir.dt.float16
F8 = mybir.dt.float8e4

N_CORES = 8
BATCH, H, W = 128, 512, 512
IMGS_PER_CORE = BATCH // N_CORES      # 16
BLK = 128
NBLK = H // BLK                       # 4
K = 11
PAD = K // 2                          # 5
ZH = K                                # zero head width
WT = ZH + PAD + W + PAD               # 533 segment width
X0 = ZH + PAD                         # x offset within segment (16)
FLAT = NBLK * WT                      # 2132
SCLEN = FLAT - ZH                     # 2121 scan steps
S0 = K - 1                            # first valid s col within a segment
ROWS = IMGS_PER_CORE * BLK            # 2048 partition-rows per core
CN = ("bm_top", "bm_mid", "bm_bot", "bhp", "bhn", "idn", "halo", "halo_lo")
EDGE = (0, IMGS_PER_CORE - 1)
LAST = IMGS_PER_CORE - 1


def _register_wscan():
    name = "ADAPT_WSCAN"
    if name in _dops._SUB_OPCODE_FOR_NAME:
        return next(o for o in _dops.OPS if o.name == name)
    spec = Spec(
        body=scan(AluOp.ADD, Src0 - Src1),
        reference=lambda in0, in1, s0, s1, imm2: np.cumsum(
            in0.astype(np.float32) - in1.astype(np.float32), axis=-1),
    )
    row = _dops._CUSTOM_DVE_ROW_BASE + len(_dops.OPS)
    _dops._SUB_OPCODE_FOR_NAME[name] = row
    shas = {}
    for ver in ("v3", "v4"):
        tmp = DveOpSpec(name=name, opcode=row, uops=lower(spec, ver=ver),
                        rd1_en=_hs1(spec))
        shas[ver] = tmp.sha(ver)
    op = _dops.DveOp(name, spec, subdim=False, uops_sha=shas)
    _dops.OPS.append(op)
    _dops.CUSTOM_DVE_SPECS[name] = spec
    return op


def _band_matrices(dtype=np.float16):
    r = np.arange(BLK)
    bm_mid = (np.abs(r[:, None] - r[None, :]) <= PAD).astype(dtype)
    bm_top = bm_mid.copy()
    for rr in range(PAD):
        bm_top[0, rr] += dtype(PAD - rr)
    bm_bot = bm_mid.copy()
    for rr in range(BLK - PAD, BLK):
        bm_bot[BLK - 1, rr] += dtype(rr - (BLK - PAD - 1))
    bhp = np.zeros((BLK, BLK), dtype=dtype)
    for p in range(BLK - PAD, BLK):
        bhp[p, 0:p - (BLK - PAD) + 1] = 1.0
    bhn = np.zeros((BLK, BLK), dtype=dtype)
    for p in range(PAD):
        bhn[p, BLK - PAD + p:BLK] = 1.0
    idn = (-121.0 * np.eye(BLK)).astype(dtype)
    # Compact-halo stationaries. halo rows 0:5 = the 5 rows just ABOVE the
    # block (rel -5..-1), rows 5:10 = the 5 rows just BELOW (rel 128..132).
    # halo_lo = the "below" weights relocated to contraction rows 0:5 (used
    # by bank 0, whose Ht chunk stores only below-rows, at partitions 0:5).
    halo = np.zeros((BLK, BLK), dtype=dtype)
    for c in range(PAD):
        halo[c, 0:c + 1] = 1.0
    for c in range(PAD, 2 * PAD):
        halo[c, c + BLK - 2 * PAD:BLK] = 1.0
    halo_lo = np.zeros((BLK, BLK), dtype=dtype)
    halo_lo[0:PAD, :] = halo[PAD:2 * PAD, :]
    return {"bm_top": bm_top, "bm_mid": bm_mid, "bm_bot": bm_bot,
            "bhp": bhp, "bhn": bhn, "idn": idn,
            "halo": halo, "halo_lo": halo_lo}


def _build():
    wop = _register_wscan()
    nc = bacc.Bacc(None, target_bir_lowering=False, debug=False)
    x_d = nc.declare_dram_parameter("x", [ROWS, FLAT], F16, isOutput=False)
    c_d = nc.declare_dram_parameter("consts", [BLK, len(CN) * BLK], F16,
                                    isOutput=False)
    out_d = nc.declare_dram_parameter("out", [ROWS, NBLK * W], F8, isOutput=True)

    with tile.TileContext(nc) as tc:
        with (
            tc.tile_pool(name="cpool", bufs=1) as cpool,
            tc.tile_pool(name="xin", bufs=5) as x_pool,
            tc.tile_pool(name="scr", bufs=5) as s_pool,
            tc.tile_pool(name="outp", bufs=3) as o_pool,
            tc.tile_pool(name="warm", bufs=1) as w_pool,
            tc.tile_pool(name="psumA", bufs=2, space=bass.MemorySpace.PSUM) as ps_pool_a,
            tc.tile_pool(name="psumB", bufs=2, space=bass.MemorySpace.PSUM) as ps_pool_b,
        ):
            cbig = cpool.tile([BLK, len(CN) * BLK], F16, tag="consts")
            nc.scalar.dma_start(cbig[:], c_d[:])
            ct = {nm: cbig[:, j * BLK:(j + 1) * BLK] for j, nm in enumerate(CN)}

            def cpart(nm, k):  # stationary slice with k contraction rows
                j = CN.index(nm)
                return cbig[0:k, j * BLK:(j + 1) * BLK]

            bias_t = cpool.tile([BLK, 1], F32, tag="bias")
            nc.vector.memset(bias_t[:], -242.0 / 4.0)

            # PE p-state warmup: data-independent matmuls into scratch psum
            # tiles ramp the PE clock during the startup preamble. They cycle
            # a full start/stop group through ALL 8 psum banks: hardware
            # lazy-zero ("pending zero") state from power-on is consumed here,
            # so later start=False accumulates onto ACT-written psum are safe.
            # ACT-init bank3s always live in the SECOND bank of a psumB-pool
            # tile (by construction: separate pools for the a/b halves). Those
            # two physical banks must see a full-width start/stop matmul group
            # before the first start=False accumulate onto ACT-written data,
            # to consume power-on lazy-zero state. idn banks start=True and
            # self-clear, so everywhere else short ramp-only warmups suffice.
            wsb = w_pool.tile([BLK, 5 * BLK], F16, tag="wsb")
            nc.gpsimd.memset(wsb[:], 0.0)
            wtiles = [ps_pool_b.tile([BLK, 2, W], F32, tag="ps",
                                     name=f"ps_warmb{wi}") for wi in range(2)]
            wtiles += [ps_pool_a.tile([BLK, 2, W], F32, tag="ps",
                                      name=f"ps_warma{wi}") for wi in range(2)]
            for half in (0, 1):  # cover all 8 banks; bridges until data-ready
                for wps in wtiles:
                    nc.tensor.matmul(wps[:, half, :], wsb[:, 0:BLK],
                                     wsb[:, BLK:], start=True, stop=True)

            imgs = {}     # i -> (ximg, s, Ht)
            psums = {}    # i -> (ps_a [banks 0,1], ps_b [banks 2,3])
            outs = {}     # i -> oimg

            def is_init_img(i):
                # bank 3's -121*y pre-load runs on the Scalar engine for every
                # image; PE accumulates on top with start=False
                return True

            def is_init2_img(i):
                # bank 2 is also ACT-pre-loaded on a third of the images:
                # uses ACT's remaining headroom to drop one more idn matmul
                return i not in EDGE and (i % 3 == 1)

            def init_banks(i):
                b = {NBLK - 1} if is_init_img(i) else set()
                if is_init2_img(i):
                    b.add(NBLK - 2)
                return b

            def front_img(i):
                ximg = x_pool.tile([BLK, NBLK, WT], F16, tag="ximg")
                xflat = ximg[:].rearrange("q p c -> q (p c)")
                xrow = x_d[i * BLK:(i + 1) * BLK, :]
                s = s_pool.tile([BLK, NBLK, WT], F16, tag="scr")
                sflat = s[:].rearrange("q p c -> q (p c)")
                if i == 0:
                    # chunked DMA + per-segment scans to shorten pipeline fill
                    xrow3 = xrow.rearrange("q (p c) -> q p c", p=NBLK)
                    for pos in range(NBLK):
                        nc.sync.dma_start(ximg[:, pos, :], xrow3[:, pos, :])
                    for pos in range(NBLK):
                        o0 = pos * WT
                        nc.vector._custom_dve(
                            wop, out=sflat[:, o0:o0 + WT - ZH],
                            in0=xflat[:, o0 + ZH:o0 + WT],
                            in1=xflat[:, o0:o0 + WT - ZH])
                elif i == LAST:
                    # single big DMA, but per-segment scans for a short drain
                    nc.sync.dma_start(xflat[:], xrow[:])
                    for pos in range(NBLK):
                        o0 = pos * WT
                        nc.vector._custom_dve(
                            wop, out=sflat[:, o0:o0 + WT - ZH],
                            in0=xflat[:, o0 + ZH:o0 + WT],
                            in1=xflat[:, o0:o0 + WT - ZH])
                else:
                    nc.sync.dma_start(xflat[:], xrow[:])
                    nc.vector._custom_dve(
                        wop, out=sflat[:, 0:SCLEN], in0=xflat[:, ZH:FLAT],
                        in1=xflat[:, 0:SCLEN])
                imgs[i] = [ximg, s, None]

            def psum_of(i):
                if i not in psums:
                    ps_a = ps_pool_a.tile([BLK, 2, W], F32, tag="ps",
                                          name=f"ps_a{i}")
                    ps_b = ps_pool_b.tile([BLK, 2, W], F32, tag="ps",
                                          name=f"ps_b{i}")
                    psums[i] = (ps_a, ps_b)
                return psums[i]

            def bank_ap(i, p):
                ps_a, ps_b = psum_of(i)
                return (ps_a if p < 2 else ps_b)[:, p % 2, :]

            def xseg(i, p):
                return imgs[i][0][:, p, X0:X0 + W]

            def sseg(i, p):
                return imgs[i][1][:, p, S0:S0 + W]

            def emit_idn(i):
                # x-only deps: runs while image i's scan is still going
                ib = init_banks(i)
                for p in range(NBLK):
                    if p in ib:
                        continue
                    nc.tensor.matmul(bank_ap(i, p), ct["idn"], xseg(i, p),
                                     start=True, stop=False)

            def emit_init(i):
                for p in sorted(init_banks(i)):
                    nc.scalar.activation(
                        bank_ap(i, p), xseg(i, p),
                        mybir.ActivationFunctionType.Copy,
                        bias=0.0, scale=-121.0)

            def emit_bm(i):
                ib = init_banks(i)
                bmn = ["bm_top", "bm_mid", "bm_mid", "bm_bot"]
                for p in range(NBLK):
                    nc.tensor.matmul(bank_ap(i, p), ct[bmn[p]], sseg(i, p),
                                     start=False, stop=False,
                                     skip_group_check=(p in ib))

            def emit_halo(i):
                # cross-block halo via bhn/bhp band matmuls (no data motion:
                # the moving operand is the neighbour block's s segment)
                ib = init_banks(i)
                for p in range(NBLK - 1):
                    # bank 0's last write is bhn_0; banks 1,2 still get bhp
                    nc.tensor.matmul(bank_ap(i, p), ct["bhn"], sseg(i, p + 1),
                                     start=False, stop=(p == 0),
                                     skip_group_check=(p in ib))
                for p in range(1, NBLK):
                    nc.tensor.matmul(bank_ap(i, p), ct["bhp"], sseg(i, p - 1),
                                     start=False, stop=True,
                                     skip_group_check=(p in ib))

            def oimg_of(i):
                if i not in outs:
                    outs[i] = o_pool.tile([BLK, NBLK, W], F8, tag="oimg",
                                          name=f"oimg{i}")
                return outs[i]

            def emit_sign(i, half, q=None, store=False):
                # half 0: banks 0,1 ; half 1: banks 2,3
                oimg = oimg_of(i)
                ps = psum_of(i)[half]
                nc.scalar.activation(
                    oimg[:, 2 * half:2 * half + 2, :], ps[:],
                    mybir.ActivationFunctionType.Sign,
                    bias=bias_t[:], scale=1.0)
                if store:
                    # store this half alone (drain path)
                    oflat = oimg[:].rearrange("q p c -> q (p c)")
                    orow = out_d[i * BLK:(i + 1) * BLK, :]
                    hw = NBLK * W // 2
                    (q or nc.scalar).dma_start(
                        orow[:, half * hw:(half + 1) * hw],
                        oflat[:, half * hw:(half + 1) * hw])

            def emit_sign_bank(i, p, q, store_half=False):
                # single-bank sign (overlaps the next bank's matmuls); the
                # store, when requested, covers the whole half in one DMA
                # with 1KB descriptors
                oimg = oimg_of(i)
                ps = psum_of(i)[p // 2]
                nc.scalar.activation(
                    oimg[:, p, :], ps[:, p % 2, :],
                    mybir.ActivationFunctionType.Sign,
                    bias=bias_t[:], scale=1.0)
                if store_half:
                    oflat = oimg[:].rearrange("q p c -> q (p c)")
                    orow = out_d[i * BLK:(i + 1) * BLK, :]
                    half = p // 2
                    hw = NBLK * W // 2
                    q.dma_start(orow[:, half * hw:(half + 1) * hw],
                                oflat[:, half * hw:(half + 1) * hw])

            def emit_out(i):
                # one big-descriptor store for the whole image, on the idle
                # gpsimd/SWDGE path: keeps both the scalar queue (sign/init)
                # and the sync queue (input + gathers) clear
                oimg = oimg_of(i)
                oflat = oimg[:].rearrange("q p c -> q (p c)")
                nc.gpsimd.dma_start(out_d[i * BLK:(i + 1) * BLK, :], oflat[:])

            def mm_edge(i, wname, bank, mv, start, stop):
                nc.tensor.matmul(bank_ap(i, bank), ct[wname], mv,
                                 start=start, stop=stop,
                                 skip_group_check=(is_init_img(i) and bank == 3))

            # ---- pipeline ----
            front_img(0)
            front_img(1)
            front_img(2)
            emit_idn(0)
            emit_init(0)

            for i in range(IMGS_PER_CORE):
                if i == 0:
                    # fill: per-segment availability order (bhn/bhp path)
                    mm_edge(i, "bm_top", 0, sseg(i, 0), False, False)
                    mm_edge(i, "bhp", 1, sseg(i, 0), False, False)
                    mm_edge(i, "bm_mid", 1, sseg(i, 1), False, False)
                    mm_edge(i, "bhn", 0, sseg(i, 1), False, True)
                    mm_edge(i, "bhp", 2, sseg(i, 1), False, False)
                    mm_edge(i, "bm_mid", 2, sseg(i, 2), False, False)
                    mm_edge(i, "bhn", 1, sseg(i, 2), False, True)
                    emit_sign(i, 0)
                    mm_edge(i, "bhp", 3, sseg(i, 2), False, False)
                    mm_edge(i, "bm_bot", 3, sseg(i, 3), False, True)
                    mm_edge(i, "bhn", 2, sseg(i, 3), False, True)
                    emit_init(i + 1)
                    emit_idn(i + 1)
                    emit_sign(i, 1)
                    emit_out(i)
                elif i == LAST:
                    # drain: banks complete 0->3; store each half asap
                    mm_edge(i, "bm_top", 0, sseg(i, 0), False, False)
                    mm_edge(i, "bhp", 1, sseg(i, 0), False, False)
                    mm_edge(i, "bhn", 0, sseg(i, 1), False, True)
                    mm_edge(i, "bm_mid", 1, sseg(i, 1), False, False)
                    mm_edge(i, "bhp", 2, sseg(i, 1), False, False)
                    mm_edge(i, "bhn", 1, sseg(i, 2), False, True)
                    emit_sign(i, 0, q=nc.sync, store=True)
                    mm_edge(i, "bm_mid", 2, sseg(i, 2), False, False)
                    mm_edge(i, "bhp", 3, sseg(i, 2), False, False)
                    mm_edge(i, "bhn", 2, sseg(i, 3), False, True)
                    mm_edge(i, "bm_bot", 3, sseg(i, 3), False, True)
                    emit_sign(i, 1, q=nc.sync, store=True)
                else:
                    emit_init(i + 1)
                    emit_bm(i)
                    emit_halo(i)
                    emit_idn(i + 1)
                    emit_sign(i, 0)
                    emit_sign(i, 1)
                    emit_out(i)
                    imgs.pop(i)
                if i + 3 < IMGS_PER_CORE:
                    front_img(i + 3)
    nc.compile()
    return nc


_NC_CACHE = None


def _make_in_maps(x: np.ndarray) -> list:
    x = np.asarray(x, dtype=np.float32)
    y = (x.reshape(BATCH, H, W).astype(np.float16) / np.float16(4.0))
    yq = y.reshape(BATCH, NBLK, BLK, W).transpose(0, 2, 1, 3)
    plane = np.zeros((BATCH, BLK, NBLK, WT), dtype=np.float16)
    plane[..., X0:X0 + W] = yq
    plane[..., ZH:X0] = yq[..., 0:1]
    plane[..., X0 + W:WT] = yq[..., W - 1:W]
    cm = _band_matrices()
    cbig = np.concatenate([cm[nm] for nm in CN], axis=1)
    in_maps = []
    for c in range(N_CORES):
        shard = plane[c * IMGS_PER_CORE:(c + 1) * IMGS_PER_CORE].reshape(
            ROWS, FLAT)
        in_maps.append({"x": np.ascontiguousarray(shard),
                        "consts": np.ascontiguousarray(cbig)})
    return in_maps


def kernel(x: np.ndarray) -> np.ndarray:
    global _NC_CACHE
    if _NC_CACHE is None:
        _NC_CACHE = _build()
    nc = _NC_CACHE
    in_maps = _make_in_maps(x)
    res = run_bass_kernel_spmd(nc, in_maps, core_ids=list(range(N_CORES)))
    out = np.empty((BATCH, H, W), dtype=np.float32)
    for c in range(N_CORES):
        sgn = np.asarray(res.results[c]["out"]).view(np.uint8)
        o = (sgn < 0x80).astype(np.float32) * np.float32(255.0)
        out[c * IMGS_PER_CORE:(c + 1) * IMGS_PER_CORE] = \
            o.reshape(IMGS_PER_CORE, BLK, NBLK, W).transpose(0, 2, 1, 3).reshape(
                IMGS_PER_CORE, H, W)
    return out.reshape(BATCH, H, W, 1)
